# revision 23
# baseline (speedup 1.0000x reference)
"""Bahdanau attention on 8 Trainium2 NeuronCores.

Sharding: batch dim B=32 split across 8 cores (4 per core); weights
replicated.  Per core:
  scores^T[b,t] = v . tanh(Wq@dec[b] + Wk@enc[t,b])   (PE + ACT)
  alpha^T      = masked softmax over t                 (DVE/ACT)
  ctx[b]       = sum_t alpha[t,b] * enc[t,b]           (PE)
enc is streamed k-major via xbar transpose-DMA (bf16) for the big
matmul, and kept resident t-major for the context pass.
"""

import os
import sys

import numpy as np

for _p in ("/opt/trn_rl_repo", "/root/.axon_site/_ro/trn_rl_repo"):
    if os.path.isdir(_p) and _p not in sys.path:
        sys.path.insert(0, _p)

import ml_dtypes  # noqa: E402

from concourse import bass, mybir  # noqa: E402
import concourse.bacc as bacc  # noqa: E402
import concourse.tile as tile  # noqa: E402
from concourse.bass_utils import run_bass_kernel_spmd  # noqa: E402

F32 = mybir.dt.float32
BF16 = mybir.dt.bfloat16
AX = mybir.AxisListType
AF = mybir.ActivationFunctionType

T, B, H, K, Q = 2048, 32, 1024, 1024, 1024
NCORES = 8
BL = B // NCORES  # 4 batch entries per core
P = 128
KC, HC, QC = K // P, H // P, Q // P  # 8 each
TC = T // P  # 16 natural t-blocks
CH = 512  # matmul moving free dim / PSUM bank
CHL = 1024  # transpose-load chunk (t rows per load)
NL = T // CHL  # 2 load groups
NCC = CHL // CH  # 2 compute chunks per load group

LAST_RESULTS = None  # BassKernelResults of the most recent run (for test.py)
_CACHED = None  # (nc, names) — build once per process


def _build():
    nc = bacc.Bacc("TRN2", target_bir_lowering=False, debug=False,
                   num_devices=NCORES)

    enc_d = nc.dram_tensor("enc", [T, BL, K], BF16, kind="ExternalInput")
    # weights pre-shuffled on host to [partition, chunk, col] so each
    # load is one contiguous run per partition (fast descriptor stream)
    wkt_d = nc.dram_tensor("wkt", [P, KC, H], BF16, kind="ExternalInput")
    wqt_d = nc.dram_tensor("wqt", [P, QC, H], BF16, kind="ExternalInput")
    dect_d = nc.dram_tensor("dect", [P, QC, BL], BF16, kind="ExternalInput")
    maskt_d = nc.dram_tensor("maskt", [BL, T], BF16, kind="ExternalInput")
    # v chunks replicated x4 so the v-dot matmul writes PSUM rows 0..3
    # (PE output base partition must be 0/32/64); row b is then copied out.
    # bf16: fp32 matmuls run in LOW_HIGH mode at ~2x the cycles.
    vt_d = nc.dram_tensor("vt", [P, HC, BL], BF16, kind="ExternalInput")
    # e416[i, 4a+j] = delta(i,j): scores.T @ e416 lays alpha out t-major,
    # replicated 4x, feeding the context matmuls' stationary operand.
    e416_d = nc.dram_tensor("e416", [BL, BL * BL], F32, kind="ExternalInput")

    alphat_d = nc.dram_tensor("alphat", [BL, T], F32, kind="ExternalOutput")
    ctx_d = nc.dram_tensor("ctx", [BL, K], F32, kind="ExternalOutput")

    with tile.TileContext(nc) as tc:
        with (
            tc.tile_pool(name="const", bufs=1) as const_pool,
            tc.tile_pool(name="wkt", bufs=1) as wkt_pool,
            tc.tile_pool(name="encres", bufs=1) as encres_pool,
            tc.tile_pool(name="enct", bufs=13) as enct_pool,
            tc.tile_pool(name="tanh", bufs=2) as tanh_pool,
            tc.tile_pool(name="small", bufs=1) as small_pool,
            tc.tile_pool(name="tmp", bufs=2) as tmp_pool,
            tc.tile_pool(name="ppk", bufs=3, space="PSUM") as ppk_pool,
            tc.tile_pool(name="ps", bufs=2, space="PSUM") as ps_pool,
            tc.tile_pool(name="pmisc", bufs=2, space="PSUM") as pmisc_pool,
        ):
            # ---- constants / weights ----
            vt_sb = const_pool.tile([P, HC, BL], BF16)
            nc.scalar.dma_start(vt_sb[:], vt_d.ap())
            e416_sb = const_pool.tile([BL, BL * BL], F32)
            nc.scalar.dma_start(e416_sb[:], e416_d.ap())
            maskt_sb = const_pool.tile([BL, T], BF16)
            nc.scalar.dma_start(maskt_sb[:], maskt_d.ap())
            dect_sb = const_pool.tile([P, QC, BL], BF16)
            nc.scalar.dma_start(dect_sb[:], dect_d.ap())
            # ---- stage 1: qT[h, b] = sum_q Wq[h,q] dec[b,q] ----
            # wqt load issued before wkt: the q matmuls are the first PE work
            qt_sb = const_pool.tile([P, HC * BL], F32)
            wkt_sb = wkt_pool.tile([P, KC, H], BF16)
            with tc.tile_pool(name="wqt", bufs=1) as wqt_pool:
                wqt_sb = wqt_pool.tile([P, QC, H], BF16)
                nc.scalar.dma_start(wqt_sb[:], wqt_d.ap())
                nc.scalar.dma_start(wkt_sb[:], wkt_d.ap())
                for hc in range(HC):
                    pq = pmisc_pool.tile([P, BL], F32, tag="pm")
                    for qc in range(QC):
                        nc.tensor.matmul(
                            pq[:],
                            wqt_sb[:, qc, hc * P:(hc + 1) * P],
                            dect_sb[:, qc, :],
                            start=(qc == 0), stop=(qc == QC - 1))
                    nc.vector.tensor_copy(
                        qt_sb[:, hc * BL:(hc + 1) * BL], pq[:])

            # resident natural-layout enc (t-major) for the context pass
            enc_sb = encres_pool.tile([P, TC, BL, K], BF16)

            # ---- stage 2: scores ----
            scores_sb = small_pool.tile([BL, T], F32)
            res_loaded = 0
            for lc in range(NL):
                for b in range(BL):
                    enct = []
                    for kc in range(KC):
                        tl = enct_pool.tile([P, CHL], BF16, tag="enct")
                        nc.sync.dma_start(
                            tl[:],
                            enc_d.ap()[lc * CHL:(lc + 1) * CHL, b,
                                       kc * P:(kc + 1) * P],
                            transpose=True)
                        enct.append(tl)
                    # spread the 16 resident-enc loads across the 8 groups;
                    # gpsimd queue so DMACopy never interleaves with the
                    # sync queue's xbar transposes (mode-switch serializes).
                    for _ in range(2):
                        if res_loaded < TC:
                            tj = res_loaded
                            nc.gpsimd.dma_start(
                                enc_sb[:, tj],
                                enc_d.ap()[tj * P:(tj + 1) * P])
                            res_loaded += 1
                    for cc in range(NCC):
                        tcix = lc * NCC + cc
                        ps = ps_pool.tile([BL, CH], F32, tag="ps")
                        for hc in range(HC):
                            pk = ppk_pool.tile([P, CH], F32, tag="pk")
                            for kc in range(KC):
                                nc.tensor.matmul(
                                    pk[:],
                                    wkt_sb[:, kc, hc * P:(hc + 1) * P],
                                    enct[kc][:, cc * CH:(cc + 1) * CH],
                                    start=(kc == 0), stop=(kc == KC - 1))
                            th = tanh_pool.tile([P, CH], BF16, tag="th")
                            nc.scalar.activation(
                                th[:], pk[:], AF.Tanh,
                                bias=qt_sb[:, hc * BL + b:hc * BL + b + 1])
                            nc.tensor.matmul(
                                ps[:], vt_sb[:, hc, :], th[:],
                                start=(hc == 0), stop=(hc == HC - 1))
                        # ps rows 0..3 all hold s_b.  Apply exp here
                        # (overlapped with stage 2); masking happens on the
                        # assembled [4, T] tile (rows differ per b!).  No
                        # max-subtraction needed: |s| <= sum|v| ~ 25 keeps
                        # exp well inside fp32 range.
                        est = tmp_pool.tile([BL, CH], F32, tag="st")
                        nc.scalar.activation(est[:], ps[:], AF.Exp)
                        # engines can't address partition base b: place the
                        # row with a small DMA instead.
                        nc.gpsimd.dma_start(
                            scores_sb[b:b + 1, tcix * CH:(tcix + 1) * CH],
                            est[0:1, :])

            # ---- stage 3: finish softmax (scores_sb holds exp(s)) ----
            nc.vector.tensor_mul(scores_sb[:], scores_sb[:], maskt_sb[:])
            zs = small_pool.tile([BL, 1], F32)
            nc.vector.reduce_sum(zs[:], scores_sb[:], axis=AX.X)
            rz = small_pool.tile([BL, 1], F32)
            nc.vector.reciprocal(rz[:], zs[:])
            # fold 1/Z into the transpose matmul's stationary operand
            rze_sb = small_pool.tile([BL, BL * BL], F32)
            nc.vector.tensor_scalar_mul(rze_sb[:], e416_sb[:], rz[:])

            # ---- stage 4: alpha -> [t, (copy, b)] layout, then ctx ----
            # pt[t, 4a+j] = alpha[j, t]; slicing [:, :, b] gives alpha_b
            # replicated in 4 columns -> matmul output rows 0..3 all equal.
            alphar_sb = small_pool.tile([P, TC, BL, BL], BF16)
            for tj in range(TC):
                pt = pmisc_pool.tile([P, BL * BL], F32, tag="pm")
                nc.tensor.matmul(pt[:], scores_sb[:, tj * P:(tj + 1) * P],
                                 rze_sb[:], start=True, stop=True)
                nc.vector.tensor_copy(
                    alphar_sb[:, tj].rearrange("p a b -> p (a b)"), pt[:])

            # alphas output (overlaps the context matmuls below)
            nc.vector.tensor_scalar_mul(scores_sb[:], scores_sb[:], rz[:])
            nc.gpsimd.dma_start(alphat_d.ap(), scores_sb[:])

            for b in range(BL):
                pca = ps_pool.tile([BL, CH], F32, tag="ps")
                pcb = ps_pool.tile([BL, CH], F32, tag="ps")
                for tj in range(TC):
                    al = alphar_sb[:, tj, :, b]
                    nc.tensor.matmul(pca[:], al, enc_sb[:, tj, b, 0:CH],
                                     start=(tj == 0), stop=(tj == TC - 1))
                    nc.tensor.matmul(pcb[:], al, enc_sb[:, tj, b, CH:K],
                                     start=(tj == 0), stop=(tj == TC - 1))
                ca = tmp_pool.tile([BL, CH], F32, tag="st")
                cb = tmp_pool.tile([BL, CH], F32, tag="st")
                nc.vector.tensor_copy(ca[:], pca[:])
                nc.vector.tensor_copy(cb[:], pcb[:])
                nc.gpsimd.dma_start(ctx_d.ap()[b:b + 1, 0:CH], ca[0:1, :])
                nc.gpsimd.dma_start(ctx_d.ap()[b:b + 1, CH:K], cb[0:1, :])

    nc.compile()
    return nc


def _get_nc():
    global _CACHED
    if _CACHED is None:
        _CACHED = _build()
    return _CACHED


def _make_in_maps(decoder_hidden, encoder_hidden, mask, Wk, Wq, v):
    bf = ml_dtypes.bfloat16
    wkt = np.ascontiguousarray(
        Wk.T.reshape(KC, P, H).transpose(1, 0, 2)).astype(bf)
    wqt = np.ascontiguousarray(
        Wq.T.reshape(QC, P, H).transpose(1, 0, 2)).astype(bf)
    vt = np.ascontiguousarray(
        np.repeat(v.reshape(HC, P).T[:, :, None], BL, axis=2)
    ).astype(bf)
    e416 = np.tile(np.eye(BL, dtype=np.float32), BL)
    in_maps = []
    for c in range(NCORES):
        b0 = c * BL
        in_maps.append({
            "enc": np.ascontiguousarray(
                encoder_hidden[:, b0:b0 + BL, :]).astype(bf),
            "wkt": wkt,
            "wqt": wqt,
            "dect": np.ascontiguousarray(
                decoder_hidden[0, b0:b0 + BL, :].T.reshape(
                    QC, P, BL).transpose(1, 0, 2)).astype(bf),
            "maskt": np.ascontiguousarray(mask[:, b0:b0 + BL].T).astype(bf),
            "vt": vt,
            "e416": e416,
        })
    return in_maps


def kernel(decoder_hidden, encoder_hidden, mask, Wk, Wq, v, *,
           trace=False, trace_kwargs=None):
    global LAST_RESULTS
    nc = _get_nc()
    in_maps = _make_in_maps(decoder_hidden, encoder_hidden, mask, Wk, Wq, v)
    res = run_bass_kernel_spmd(nc, in_maps, core_ids=list(range(NCORES)),
                               trace=trace, **(trace_kwargs or {}))
    LAST_RESULTS = res
    ctx = np.concatenate([res.results[c]["ctx"] for c in range(NCORES)],
                         axis=0)[None, :, :].astype(np.float32)
    alphas = np.concatenate(
        [np.asarray(res.results[c]["alphat"]).T for c in range(NCORES)],
        axis=1).astype(np.float32)
    return ctx, alphas


# revision 29
# speedup vs baseline: 1.1427x; 1.1427x over previous
"""Bahdanau attention on 8 Trainium2 NeuronCores.

Sharding: batch dim B=32 split across 8 cores (4 per core); weights
replicated.  Per core:
  scores^T[b,t] = v . tanh(Wq@dec[b] + Wk@enc[t,b])   (PE + ACT)
  alpha^T      = masked softmax over t                 (DVE/ACT)
  ctx[b]       = sum_t alpha[t,b] * enc[t,b]           (PE)
enc is streamed k-major via xbar transpose-DMA (bf16) for the big
matmul, and kept resident t-major for the context pass.
"""

import os
import sys

import numpy as np

for _p in ("/opt/trn_rl_repo", "/root/.axon_site/_ro/trn_rl_repo"):
    if os.path.isdir(_p) and _p not in sys.path:
        sys.path.insert(0, _p)

import ml_dtypes  # noqa: E402

from concourse import bass, mybir  # noqa: E402
import concourse.bacc as bacc  # noqa: E402
import concourse.tile as tile  # noqa: E402
from concourse.bass_utils import run_bass_kernel_spmd  # noqa: E402

F32 = mybir.dt.float32
BF16 = mybir.dt.bfloat16
AX = mybir.AxisListType
AF = mybir.ActivationFunctionType

T, B, H, K, Q = 2048, 32, 1024, 1024, 1024
NCORES = 8
BL = B // NCORES  # 4 batch entries per core
P = 128
KC, HC, QC = K // P, H // P, Q // P  # 8 each
TC = T // P  # 16 natural t-blocks
CH = 512  # matmul moving free dim / PSUM bank
CHL = 1024  # transpose-load chunk (t rows per load)
NL = T // CHL  # 2 load groups
NCC = CHL // CH  # 2 compute chunks per load group

LAST_RESULTS = None  # BassKernelResults of the most recent run (for test.py)
_CACHED = None  # (nc, names) — build once per process


def _build():
    nc = bacc.Bacc("TRN2", target_bir_lowering=False, debug=False,
                   num_devices=NCORES)

    enc_d = nc.dram_tensor("enc", [T, BL, K], BF16, kind="ExternalInput")
    # same data k-major (host-marshalled): stream loads become contiguous
    # plain DMAs; the xbar transpose path runs at ~50 GB/s on 256B rows.
    enct_d = nc.dram_tensor("enct", [BL, K, T], BF16, kind="ExternalInput")
    # weights pre-shuffled on host to [partition, chunk, col] so each
    # load is one contiguous run per partition (fast descriptor stream)
    wkt_d = nc.dram_tensor("wkt", [P, KC, H], BF16, kind="ExternalInput")
    wqt_d = nc.dram_tensor("wqt", [P, QC, H], BF16, kind="ExternalInput")
    dect_d = nc.dram_tensor("dect", [P, QC, BL], BF16, kind="ExternalInput")
    maskt_d = nc.dram_tensor("maskt", [BL, T], BF16, kind="ExternalInput")
    # v chunks replicated x4 so the v-dot matmul writes PSUM rows 0..3
    # (PE output base partition must be 0/32/64); row b is then copied out.
    # bf16: fp32 matmuls run in LOW_HIGH mode at ~2x the cycles.
    vt_d = nc.dram_tensor("vt", [P, HC, BL], BF16, kind="ExternalInput")
    # e416[i, 4a+j] = delta(i,j): scores.T @ e416 lays alpha out t-major,
    # replicated 4x, feeding the context matmuls' stationary operand.
    e416_d = nc.dram_tensor("e416", [BL, BL * BL], F32, kind="ExternalInput")

    alphat_d = nc.dram_tensor("alphat", [BL, T], F32, kind="ExternalOutput")
    ctx_d = nc.dram_tensor("ctx", [BL, K], F32, kind="ExternalOutput")

    with tile.TileContext(nc) as tc:
        with (
            tc.tile_pool(name="const", bufs=1) as const_pool,
            tc.tile_pool(name="wkt", bufs=1) as wkt_pool,
            tc.tile_pool(name="encres", bufs=1) as encres_pool,
            tc.tile_pool(name="enct", bufs=3) as enct_pool,
            tc.tile_pool(name="tanh", bufs=2) as tanh_pool,
            tc.tile_pool(name="small", bufs=1) as small_pool,
            tc.tile_pool(name="tmp", bufs=2) as tmp_pool,
            tc.tile_pool(name="ppk", bufs=3, space="PSUM") as ppk_pool,
            tc.tile_pool(name="ps", bufs=2, space="PSUM") as ps_pool,
            tc.tile_pool(name="pmisc", bufs=2, space="PSUM") as pmisc_pool,
        ):
            # ---- constants / weights ----
            vt_sb = const_pool.tile([P, HC, BL], BF16)
            nc.scalar.dma_start(vt_sb[:], vt_d.ap())
            e416_sb = const_pool.tile([BL, BL * BL], F32)
            nc.scalar.dma_start(e416_sb[:], e416_d.ap())
            maskt_sb = const_pool.tile([BL, T], BF16)
            nc.scalar.dma_start(maskt_sb[:], maskt_d.ap())
            dect_sb = const_pool.tile([P, QC, BL], BF16)
            nc.scalar.dma_start(dect_sb[:], dect_d.ap())
            # ---- stage 1: qT[h, b] = sum_q Wq[h,q] dec[b,q] ----
            # wqt load issued before wkt: the q matmuls are the first PE work
            qt_sb = const_pool.tile([P, HC * BL], F32)
            wkt_sb = wkt_pool.tile([P, KC, H], BF16)
            with tc.tile_pool(name="wqt", bufs=1) as wqt_pool:
                wqt_sb = wqt_pool.tile([P, QC, H], BF16)
                nc.scalar.dma_start(wqt_sb[:], wqt_d.ap())
                nc.scalar.dma_start(wkt_sb[:], wkt_d.ap())
                for hc in range(HC):
                    pq = pmisc_pool.tile([P, BL], F32, tag="pm")
                    for qc in range(QC):
                        nc.tensor.matmul(
                            pq[:],
                            wqt_sb[:, qc, hc * P:(hc + 1) * P],
                            dect_sb[:, qc, :],
                            start=(qc == 0), stop=(qc == QC - 1))
                    nc.vector.tensor_copy(
                        qt_sb[:, hc * BL:(hc + 1) * BL], pq[:])

            # resident natural-layout enc (t-major) for the context pass
            enc_sb = encres_pool.tile([P, TC, BL, K], BF16)

            # ---- stage 2: scores ----
            scores_sb = small_pool.tile([BL, T], F32)
            res_loaded = 0
            for lc in range(NL):
                for b in range(BL):
                    # spread the 16 resident-enc loads across the 8 groups;
                    # gpsimd queue so DMACopy never interleaves with the
                    # sync queue's xbar transposes (mode-switch serializes).
                    for _ in range(2):
                        if res_loaded < TC:
                            tj = res_loaded
                            nc.gpsimd.dma_start(
                                enc_sb[:, tj],
                                enc_d.ap()[tj * P:(tj + 1) * P])
                            res_loaded += 1
                    for cc in range(NCC):
                        tcix = lc * NCC + cc
                        # k-major stream chunk, split across the two HWDGE
                        # queues (contiguous 1KB runs per partition)
                        src = enct_d.ap()[b, :, tcix * CH:(tcix + 1) * CH]
                        src = src.rearrange("(kc p) t -> p kc t", p=P)
                        tl = enct_pool.tile([P, KC, CH], BF16, tag="enct")
                        nc.sync.dma_start(tl[:, 0:KC // 2], src[:, 0:KC // 2])
                        nc.scalar.dma_start(tl[:, KC // 2:], src[:, KC // 2:])
                        ps = ps_pool.tile([BL, CH], F32, tag="ps")
                        for hc in range(HC):
                            pk = ppk_pool.tile([P, CH], F32, tag="pk")
                            for kc in range(KC):
                                nc.tensor.matmul(
                                    pk[:],
                                    wkt_sb[:, kc, hc * P:(hc + 1) * P],
                                    tl[:, kc, :],
                                    start=(kc == 0), stop=(kc == KC - 1))
                            th = tanh_pool.tile([P, CH], BF16, tag="th")
                            nc.scalar.activation(
                                th[:], pk[:], AF.Tanh,
                                bias=qt_sb[:, hc * BL + b:hc * BL + b + 1])
                            nc.tensor.matmul(
                                ps[:], vt_sb[:, hc, :], th[:],
                                start=(hc == 0), stop=(hc == HC - 1))
                        # ps rows 0..3 all hold s_b.  Apply exp here
                        # (overlapped with stage 2); masking happens on the
                        # assembled [4, T] tile (rows differ per b!).  No
                        # max-subtraction needed: |s| <= sum|v| ~ 25 keeps
                        # exp well inside fp32 range.
                        est = tmp_pool.tile([BL, CH], F32, tag="st")
                        nc.scalar.activation(est[:], ps[:], AF.Exp)
                        # engines can't address partition base b: place the
                        # row with a small DMA instead.
                        nc.gpsimd.dma_start(
                            scores_sb[b:b + 1, tcix * CH:(tcix + 1) * CH],
                            est[0:1, :])

            # ---- stage 3: finish softmax (scores_sb holds exp(s)) ----
            nc.vector.tensor_mul(scores_sb[:], scores_sb[:], maskt_sb[:])
            zs = small_pool.tile([BL, 1], F32)
            nc.vector.reduce_sum(zs[:], scores_sb[:], axis=AX.X)
            rz = small_pool.tile([BL, 1], F32)
            nc.vector.reciprocal(rz[:], zs[:])
            # fold 1/Z into the transpose matmul's stationary operand
            rze_sb = small_pool.tile([BL, BL * BL], F32)
            nc.vector.tensor_scalar_mul(rze_sb[:], e416_sb[:], rz[:])

            # ---- stage 4: alpha -> [t, (copy, b)] layout, then ctx ----
            # pt[t, 4a+j] = alpha[j, t]; slicing [:, :, b] gives alpha_b
            # replicated in 4 columns -> matmul output rows 0..3 all equal.
            alphar_sb = small_pool.tile([P, TC, BL, BL], BF16)
            for tj in range(TC):
                pt = pmisc_pool.tile([P, BL * BL], F32, tag="pm")
                nc.tensor.matmul(pt[:], scores_sb[:, tj * P:(tj + 1) * P],
                                 rze_sb[:], start=True, stop=True)
                nc.vector.tensor_copy(
                    alphar_sb[:, tj].rearrange("p a b -> p (a b)"), pt[:])

            # alphas output (overlaps the context matmuls below)
            nc.vector.tensor_scalar_mul(scores_sb[:], scores_sb[:], rz[:])
            nc.gpsimd.dma_start(alphat_d.ap(), scores_sb[:])

            for b in range(BL):
                pca = ps_pool.tile([BL, CH], F32, tag="ps")
                pcb = ps_pool.tile([BL, CH], F32, tag="ps")
                for tj in range(TC):
                    al = alphar_sb[:, tj, :, b]
                    nc.tensor.matmul(pca[:], al, enc_sb[:, tj, b, 0:CH],
                                     start=(tj == 0), stop=(tj == TC - 1))
                    nc.tensor.matmul(pcb[:], al, enc_sb[:, tj, b, CH:K],
                                     start=(tj == 0), stop=(tj == TC - 1))
                ca = tmp_pool.tile([BL, CH], F32, tag="st")
                cb = tmp_pool.tile([BL, CH], F32, tag="st")
                nc.vector.tensor_copy(ca[:], pca[:])
                nc.vector.tensor_copy(cb[:], pcb[:])
                nc.gpsimd.dma_start(ctx_d.ap()[b:b + 1, 0:CH], ca[0:1, :])
                nc.gpsimd.dma_start(ctx_d.ap()[b:b + 1, CH:K], cb[0:1, :])

    nc.compile()
    return nc


def _get_nc():
    global _CACHED
    if _CACHED is None:
        _CACHED = _build()
    return _CACHED


def _make_in_maps(decoder_hidden, encoder_hidden, mask, Wk, Wq, v):
    bf = ml_dtypes.bfloat16
    wkt = np.ascontiguousarray(
        Wk.T.reshape(KC, P, H).transpose(1, 0, 2)).astype(bf)
    wqt = np.ascontiguousarray(
        Wq.T.reshape(QC, P, H).transpose(1, 0, 2)).astype(bf)
    vt = np.ascontiguousarray(
        np.repeat(v.reshape(HC, P).T[:, :, None], BL, axis=2)
    ).astype(bf)
    e416 = np.tile(np.eye(BL, dtype=np.float32), BL)
    in_maps = []
    for c in range(NCORES):
        b0 = c * BL
        in_maps.append({
            "enc": np.ascontiguousarray(
                encoder_hidden[:, b0:b0 + BL, :]).astype(bf),
            "enct": np.ascontiguousarray(
                encoder_hidden[:, b0:b0 + BL, :].astype(bf)
                .transpose(1, 2, 0)),
            "wkt": wkt,
            "wqt": wqt,
            "dect": np.ascontiguousarray(
                decoder_hidden[0, b0:b0 + BL, :].T.reshape(
                    QC, P, BL).transpose(1, 0, 2)).astype(bf),
            "maskt": np.ascontiguousarray(mask[:, b0:b0 + BL].T).astype(bf),
            "vt": vt,
            "e416": e416,
        })
    return in_maps


def kernel(decoder_hidden, encoder_hidden, mask, Wk, Wq, v, *,
           trace=False, trace_kwargs=None):
    global LAST_RESULTS
    nc = _get_nc()
    in_maps = _make_in_maps(decoder_hidden, encoder_hidden, mask, Wk, Wq, v)
    res = run_bass_kernel_spmd(nc, in_maps, core_ids=list(range(NCORES)),
                               trace=trace, **(trace_kwargs or {}))
    LAST_RESULTS = res
    ctx = np.concatenate([res.results[c]["ctx"] for c in range(NCORES)],
                         axis=0)[None, :, :].astype(np.float32)
    alphas = np.concatenate(
        [np.asarray(res.results[c]["alphat"]).T for c in range(NCORES)],
        axis=1).astype(np.float32)
    return ctx, alphas


# revision 32
# speedup vs baseline: 1.1519x; 1.0080x over previous
"""Bahdanau attention on 8 Trainium2 NeuronCores.

Sharding: batch dim B=32 split across 8 cores (4 per core); weights
replicated.  Per core:
  scores^T[b,t] = v . tanh(Wq@dec[b] + Wk@enc[t,b])   (PE + ACT)
  alpha^T      = masked softmax over t                 (DVE/ACT)
  ctx[b]       = sum_t alpha[t,b] * enc[t,b]           (PE)
enc is streamed k-major via xbar transpose-DMA (bf16) for the big
matmul, and kept resident t-major for the context pass.
"""

import os
import sys

import numpy as np

for _p in ("/opt/trn_rl_repo", "/root/.axon_site/_ro/trn_rl_repo"):
    if os.path.isdir(_p) and _p not in sys.path:
        sys.path.insert(0, _p)

import ml_dtypes  # noqa: E402

from concourse import bass, mybir  # noqa: E402
import concourse.bacc as bacc  # noqa: E402
import concourse.tile as tile  # noqa: E402
from concourse.bass_utils import run_bass_kernel_spmd  # noqa: E402

F32 = mybir.dt.float32
BF16 = mybir.dt.bfloat16
AX = mybir.AxisListType
AF = mybir.ActivationFunctionType

T, B, H, K, Q = 2048, 32, 1024, 1024, 1024
NCORES = 8
BL = B // NCORES  # 4 batch entries per core
P = 128
KC, HC, QC = K // P, H // P, Q // P  # 8 each
TC = T // P  # 16 natural t-blocks
CH = 512  # matmul moving free dim / PSUM bank
CHL = 1024  # transpose-load chunk (t rows per load)
NL = T // CHL  # 2 load groups
NCC = CHL // CH  # 2 compute chunks per load group

LAST_RESULTS = None  # BassKernelResults of the most recent run (for test.py)
_CACHED = None  # (nc, names) — build once per process


def _build():
    nc = bacc.Bacc("TRN2", target_bir_lowering=False, debug=False,
                   num_devices=NCORES)

    enc_d = nc.dram_tensor("enc", [T, BL, K], BF16, kind="ExternalInput")
    # same data k-major (host-marshalled): stream loads become contiguous
    # plain DMAs; the xbar transpose path runs at ~50 GB/s on 256B rows.
    enct_d = nc.dram_tensor("enct", [BL, K, T], BF16, kind="ExternalInput")
    # weights pre-shuffled on host to [partition, chunk, col] so each
    # load is one contiguous run per partition (fast descriptor stream)
    wkt_d = nc.dram_tensor("wkt", [P, KC, H], BF16, kind="ExternalInput")
    wqt_d = nc.dram_tensor("wqt", [P, QC, H], BF16, kind="ExternalInput")
    dect_d = nc.dram_tensor("dect", [P, QC, BL], BF16, kind="ExternalInput")
    maskt_d = nc.dram_tensor("maskt", [BL, T], BF16, kind="ExternalInput")
    # v chunks replicated x4 so the v-dot matmul writes PSUM rows 0..3
    # (PE output base partition must be 0/32/64); row b is then copied out.
    # bf16: fp32 matmuls run in LOW_HIGH mode at ~2x the cycles.
    vt_d = nc.dram_tensor("vt", [P, HC, BL], BF16, kind="ExternalInput")
    # e416[i, 4a+j] = delta(i,j): scores.T @ e416 lays alpha out t-major,
    # replicated 4x, feeding the context matmuls' stationary operand.
    e416_d = nc.dram_tensor("e416", [BL, BL * BL], F32, kind="ExternalInput")

    alphat_d = nc.dram_tensor("alphat", [BL, T], F32, kind="ExternalOutput")
    ctx_d = nc.dram_tensor("ctx", [BL, K], F32, kind="ExternalOutput")

    with tile.TileContext(nc) as tc:
        with (
            tc.tile_pool(name="const", bufs=1) as const_pool,
            tc.tile_pool(name="wkt", bufs=1) as wkt_pool,
            tc.tile_pool(name="encres", bufs=1) as encres_pool,
            tc.tile_pool(name="enct", bufs=3) as enct_pool,
            tc.tile_pool(name="tanh", bufs=2) as tanh_pool,
            tc.tile_pool(name="small", bufs=1) as small_pool,
            tc.tile_pool(name="tmp", bufs=2) as tmp_pool,
            tc.tile_pool(name="ppk", bufs=3, space="PSUM") as ppk_pool,
            tc.tile_pool(name="ps", bufs=2, space="PSUM") as ps_pool,
            tc.tile_pool(name="pmisc", bufs=2, space="PSUM") as pmisc_pool,
        ):
            # ---- constants / weights ----
            vt_sb = const_pool.tile([P, HC, BL], BF16)
            nc.scalar.dma_start(vt_sb[:], vt_d.ap())
            e416_sb = const_pool.tile([BL, BL * BL], F32)
            nc.scalar.dma_start(e416_sb[:], e416_d.ap())
            maskt_sb = const_pool.tile([BL, T], BF16)
            nc.scalar.dma_start(maskt_sb[:], maskt_d.ap())
            dect_sb = const_pool.tile([P, QC, BL], BF16)
            nc.scalar.dma_start(dect_sb[:], dect_d.ap())
            # ---- stage 1: q = dec @ Wq^T, then transpose to qT[h, b] ----
            # dec^T as the stationary operand (4-col weight loads are free);
            # weight loads split across both HWDGE queues.
            qt_sb = const_pool.tile([P, HC * BL], F32)
            wkt_sb = wkt_pool.tile([P, KC, H], BF16)
            with tc.tile_pool(name="wqt", bufs=1) as wqt_pool:
                wqt_sb = wqt_pool.tile([P, QC, H], BF16)
                nc.sync.dma_start(wqt_sb[:, 0:QC // 2], wqt_d.ap()[:, 0:QC // 2])
                nc.scalar.dma_start(wqt_sb[:, QC // 2:], wqt_d.ap()[:, QC // 2:])
                for kc in range(KC):
                    eng = nc.sync if kc % 2 == 0 else nc.scalar
                    eng.dma_start(wkt_sb[:, kc], wkt_d.ap()[:, kc])
                for half in range(2):
                    pq = pmisc_pool.tile([BL, CH], F32, tag="pm")
                    for qc in range(QC):
                        nc.tensor.matmul(
                            pq[:], dect_sb[:, qc, :],
                            wqt_sb[:, qc, half * CH:(half + 1) * CH],
                            start=(qc == 0), stop=(qc == QC - 1))
                    qbh = tmp_pool.tile([BL, CH], F32, tag="st")
                    nc.vector.tensor_copy(qbh[:], pq[:])
                    for hcl in range(HC // 2):
                        hc = half * (HC // 2) + hcl
                        ptq = pmisc_pool.tile([P, BL], F32, tag="pm")
                        nc.tensor.matmul(
                            ptq[:], qbh[:, hcl * P:(hcl + 1) * P],
                            e416_sb[:, 0:BL], start=True, stop=True)
                        nc.vector.tensor_copy(
                            qt_sb[:, hc * BL:(hc + 1) * BL], ptq[:])

            # resident natural-layout enc (t-major) for the context pass
            enc_sb = encres_pool.tile([P, TC, BL, K], BF16)

            # ---- stage 2: scores ----
            scores_sb = small_pool.tile([BL, T], F32)
            res_loaded = 0
            for lc in range(NL):
                for b in range(BL):
                    # spread the 16 resident-enc loads across the 8 groups;
                    # gpsimd queue so DMACopy never interleaves with the
                    # sync queue's xbar transposes (mode-switch serializes).
                    for _ in range(2):
                        if res_loaded < TC:
                            tj = res_loaded
                            nc.gpsimd.dma_start(
                                enc_sb[:, tj],
                                enc_d.ap()[tj * P:(tj + 1) * P])
                            res_loaded += 1
                    for cc in range(NCC):
                        tcix = lc * NCC + cc
                        # k-major stream chunk, split across the two HWDGE
                        # queues (contiguous 1KB runs per partition)
                        src = enct_d.ap()[b, :, tcix * CH:(tcix + 1) * CH]
                        src = src.rearrange("(kc p) t -> p kc t", p=P)
                        tl = enct_pool.tile([P, KC, CH], BF16, tag="enct")
                        nc.sync.dma_start(tl[:, 0:KC // 2], src[:, 0:KC // 2])
                        nc.scalar.dma_start(tl[:, KC // 2:], src[:, KC // 2:])
                        ps = ps_pool.tile([BL, CH], F32, tag="ps")
                        for hc in range(HC):
                            pk = ppk_pool.tile([P, CH], F32, tag="pk")
                            for kc in range(KC):
                                nc.tensor.matmul(
                                    pk[:],
                                    wkt_sb[:, kc, hc * P:(hc + 1) * P],
                                    tl[:, kc, :],
                                    start=(kc == 0), stop=(kc == KC - 1))
                            th = tanh_pool.tile([P, CH], BF16, tag="th")
                            nc.scalar.activation(
                                th[:], pk[:], AF.Tanh,
                                bias=qt_sb[:, hc * BL + b:hc * BL + b + 1])
                            nc.tensor.matmul(
                                ps[:], vt_sb[:, hc, :], th[:],
                                start=(hc == 0), stop=(hc == HC - 1))
                        # ps rows 0..3 all hold s_b.  Apply exp here
                        # (overlapped with stage 2); masking happens on the
                        # assembled [4, T] tile (rows differ per b!).  No
                        # max-subtraction needed: |s| <= sum|v| ~ 25 keeps
                        # exp well inside fp32 range.
                        est = tmp_pool.tile([BL, CH], F32, tag="st")
                        nc.scalar.activation(est[:], ps[:], AF.Exp)
                        # engines can't address partition base b: place the
                        # row with a small DMA instead.
                        nc.gpsimd.dma_start(
                            scores_sb[b:b + 1, tcix * CH:(tcix + 1) * CH],
                            est[0:1, :])

            # ---- stage 3: finish softmax (scores_sb holds exp(s)) ----
            nc.vector.tensor_mul(scores_sb[:], scores_sb[:], maskt_sb[:])
            zs = small_pool.tile([BL, 1], F32)
            nc.vector.reduce_sum(zs[:], scores_sb[:], axis=AX.X)
            rz = small_pool.tile([BL, 1], F32)
            nc.vector.reciprocal(rz[:], zs[:])
            # fold 1/Z into the transpose matmul's stationary operand
            rze_sb = small_pool.tile([BL, BL * BL], F32)
            nc.vector.tensor_scalar_mul(rze_sb[:], e416_sb[:], rz[:])

            # ---- stage 4: alpha -> [t, (copy, b)] layout fused into ctx ----
            # pt[t, 4a+j] = alpha[j, t]/Z_j; slicing [:, :, b] gives alpha_b
            # replicated in 4 columns -> matmul output rows 0..3 all equal.
            # Two k-half passes so the 4 per-b accumulators + transpose tile
            # fit in PSUM; pass 1 interleaves the 16 transpose matmuls so PE
            # never idles long enough to drop out of the HAM fast state.
            alphar_sb = small_pool.tile([P, TC, BL, BL], BF16)
            acc = {}
            for b in range(BL):
                pool = ps_pool if b < 2 else ppk_pool
                acc[b] = pool.tile([BL, CH], F32, name=f"acc{b}",
                                   tag="ps" if b < 2 else "pk")
            for tj in range(TC):
                pt = pmisc_pool.tile([P, BL * BL], F32, tag="pm")
                nc.tensor.matmul(pt[:], scores_sb[:, tj * P:(tj + 1) * P],
                                 rze_sb[:], start=True, stop=True)
                nc.vector.tensor_copy(
                    alphar_sb[:, tj].rearrange("p a b -> p (a b)"), pt[:])
                for b in range(BL):
                    nc.tensor.matmul(acc[b][:], alphar_sb[:, tj, :, b],
                                     enc_sb[:, tj, b, 0:CH],
                                     start=(tj == 0), stop=(tj == TC - 1))
            # alphas output (overlaps the remaining context matmuls)
            nc.vector.tensor_scalar_mul(scores_sb[:], scores_sb[:], rz[:])
            nc.gpsimd.dma_start(alphat_d.ap(), scores_sb[:])
            for b in range(BL):
                ca = tmp_pool.tile([BL, CH], F32, tag="st")
                nc.vector.tensor_copy(ca[:], acc[b][:])
                nc.gpsimd.dma_start(ctx_d.ap()[b:b + 1, 0:CH], ca[0:1, :])
            acc2 = {}
            for b in range(BL):
                pool = ps_pool if b < 2 else ppk_pool
                acc2[b] = pool.tile([BL, CH], F32, name=f"acc2{b}",
                                    tag="ps" if b < 2 else "pk")
            for tj in range(TC):
                for b in range(BL):
                    nc.tensor.matmul(acc2[b][:], alphar_sb[:, tj, :, b],
                                     enc_sb[:, tj, b, CH:K],
                                     start=(tj == 0), stop=(tj == TC - 1))
            for b in range(BL):
                cb = tmp_pool.tile([BL, CH], F32, tag="st")
                nc.vector.tensor_copy(cb[:], acc2[b][:])
                nc.gpsimd.dma_start(ctx_d.ap()[b:b + 1, CH:K], cb[0:1, :])

    nc.compile()
    return nc


def _get_nc():
    global _CACHED
    if _CACHED is None:
        _CACHED = _build()
    return _CACHED


def _make_in_maps(decoder_hidden, encoder_hidden, mask, Wk, Wq, v):
    bf = ml_dtypes.bfloat16
    wkt = np.ascontiguousarray(
        Wk.T.reshape(KC, P, H).transpose(1, 0, 2)).astype(bf)
    wqt = np.ascontiguousarray(
        Wq.T.reshape(QC, P, H).transpose(1, 0, 2)).astype(bf)
    vt = np.ascontiguousarray(
        np.repeat(v.reshape(HC, P).T[:, :, None], BL, axis=2)
    ).astype(bf)
    e416 = np.tile(np.eye(BL, dtype=np.float32), BL)
    in_maps = []
    for c in range(NCORES):
        b0 = c * BL
        in_maps.append({
            "enc": np.ascontiguousarray(
                encoder_hidden[:, b0:b0 + BL, :]).astype(bf),
            "enct": np.ascontiguousarray(
                encoder_hidden[:, b0:b0 + BL, :].astype(bf)
                .transpose(1, 2, 0)),
            "wkt": wkt,
            "wqt": wqt,
            "dect": np.ascontiguousarray(
                decoder_hidden[0, b0:b0 + BL, :].T.reshape(
                    QC, P, BL).transpose(1, 0, 2)).astype(bf),
            "maskt": np.ascontiguousarray(mask[:, b0:b0 + BL].T).astype(bf),
            "vt": vt,
            "e416": e416,
        })
    return in_maps


def kernel(decoder_hidden, encoder_hidden, mask, Wk, Wq, v, *,
           trace=False, trace_kwargs=None):
    global LAST_RESULTS
    nc = _get_nc()
    in_maps = _make_in_maps(decoder_hidden, encoder_hidden, mask, Wk, Wq, v)
    res = run_bass_kernel_spmd(nc, in_maps, core_ids=list(range(NCORES)),
                               trace=trace, **(trace_kwargs or {}))
    LAST_RESULTS = res
    ctx = np.concatenate([res.results[c]["ctx"] for c in range(NCORES)],
                         axis=0)[None, :, :].astype(np.float32)
    alphas = np.concatenate(
        [np.asarray(res.results[c]["alphat"]).T for c in range(NCORES)],
        axis=1).astype(np.float32)
    return ctx, alphas
